# revision 2
# baseline (speedup 1.0000x reference)
# Trainium2 Bass kernel for the KerasLMU problem — v5 (v3 + drip/pre-loop tuning).
#
# Math: per time step t (T=1024),
#   u_t = x_t @ e_x                       (B,1)
#   m_t = m_{t-1} @ A.T + b_row * u_t     (B,256)   -- linear recurrence
#   h_t = lrelu(x_t @ W_x + h_{t-1} @ W_h.T + m_t @ W_m.T)
#
# m is folded into a causal convolution with precomputed G[k] = W_m @ (A^k b),
# so c_t := x_t @ W_x + m_t @ W_m.T is parallel work; only
# h_t = lrelu(c_t + h_{t-1} @ W_h.T) is sequential.
#
# v3 vs v2: the h-loop (and the dripped conv) runs inside a tile_critical
# section with hand-placed semaphores:
#   - only the last matmul of each step carries a sem update (v2 paid ~15ns
#     of PE sequencer time per matmul for the auto queue-counter updates);
#   - the c base lands in PSUM via 4 identity matmuls (start=True) at the
#     head of each step instead of a DVE cast, so the whole step is PE-only
#     and bank reuse is ordered for free (step t-1's wait on S_act>=t-1 plus
#     the in-order Scalar queue implies ACT(t-NB) completed);
#   - the full h output stays resident in SBUF (64KB/partition); 4 block
#     DMAs drain it asynchronously.
#
# Sharding: data-parallel over batch. 64 rows -> 8 cores x 8. No collectives.

import os
import sys
from contextlib import ExitStack

sys.path.insert(0, "/opt/trn_rl_repo")

import numpy as np
import ml_dtypes

import concourse.bass as bass
import concourse.tile as tile
from concourse import bacc, mybir
from concourse.bass_utils import run_bass_kernel_spmd

F32 = mybir.dt.float32
BF16 = mybir.dt.bfloat16
BF = ml_dtypes.bfloat16

NCORES = 8
BATCH = 64
BC = BATCH // NCORES          # batch rows per core = 8
FEAT = 128
HID = 512
ORDER = 256
TFULL = 1024
TBLK = 256                    # steps per output DMA block

NB = 4                        # h-loop PSUM banks (rotating)
DRIP_BUDGET = 330.0           # ns of conv matmul work dripped per h step

last_run_info = {}


def _dap(handle, offset, dims):
    base = handle[:]
    return bass.AP(tensor=base.tensor, offset=offset, ap=[list(d) for d in dims])


def build_nc(T=TFULL, tblk=TBLK):
    assert T % tblk == 0
    BT = BC * T
    KCN = T // 128

    nc = bacc.Bacc(None, target_bir_lowering=False)
    xt_d = nc.declare_dram_parameter("xt", [FEAT, BT], BF16, isOutput=False)
    whT_d = nc.declare_dram_parameter("whT", [HID, HID], BF16, isOutput=False)
    g_d = nc.declare_dram_parameter("g", [T, HID], BF16, isOutput=False)
    wx_d = nc.declare_dram_parameter("wx", [FEAT, HID], BF16, isOutput=False)
    ex_d = nc.declare_dram_parameter("ex", [FEAT, 1], BF16, isOutput=False)
    eye_d = nc.declare_dram_parameter("eye", [128, 128], BF16, isOutput=False)
    # raw h dump: [p, t, mc, b] — contiguous per partition
    out_d = nc.declare_dram_parameter("out", [128, T * 4 * BC], BF16,
                                      isOutput=True)

    UPADW = 512 + T
    upad_d = nc.dram_tensor("u_pad", [BC, UPADW], BF16)
    USHW = T + 384

    # conv tau tiles: (0,32) runs inline pre-loop; the rest drip into the
    # h-loop's PE idle window. All drip tiles <=256 cols so a single drip
    # matmul never exceeds the per-step shadow.
    INLINE_TILE = (0, 32)
    DRIP_TILES = [(32, 32), (64, 64), (128, 128), (256, 256),
                  (512, 256), (768, 256)]

    with tile.TileContext(nc) as tc:
        with tc.tile_pool(name="consts", bufs=1) as consts:
            # ---- resident constants (DMA order: ex/xT first — phase B
            # depends on them; g last, only needed by inline conv) --------
            ex_sb = consts.tile([128, 1], BF16)
            nc.sync.dma_start(out=ex_sb, in_=ex_d[:, :])
            xT_sb = consts.tile([128, BT], BF16)      # x.T : [feat, (b,tau)]
            for b8 in range(BC):
                nc.sync.dma_start(out=xT_sb[:, b8 * T:(b8 + 1) * T],
                                  in_=xt_d[:, b8 * T:(b8 + 1) * T])
            wx_sb = consts.tile([128, HID], BF16)
            nc.sync.dma_start(out=wx_sb, in_=wx_d[:, :])
            g_sb = consts.tile([128, KCN, HID], BF16)
            for kc in range(KCN):
                nc.sync.dma_start(out=g_sb[:, kc, :],
                                  in_=g_d[kc * 128:(kc + 1) * 128, :])
            eye_sb = consts.tile([128, 128], BF16)
            nc.sync.dma_start(out=eye_sb, in_=eye_d[:, :])
            whT_sb = consts.tile([128, 4, HID], BF16)
            for kc in range(4):
                nc.sync.dma_start(out=whT_sb[:, kc, :],
                                  in_=whT_d[kc * 128:(kc + 1) * 128, :])

            ushr = consts.tile([128, BC, USHW], BF16)  # reversed u shifts
            c_sb = consts.tile([128, 4, T, BC], BF16)  # c.T resident
            hb = consts.tile([128, T, 4, BC], BF16)    # full h output resident
            zrow = consts.tile([1, 512], BF16)
            nc.vector.memset(zrow, 0.0)
            h0 = consts.tile([128, 4, BC], BF16)
            nc.vector.memset(h0, 0.0)
            scratch = consts.tile([128, 4, BC], BF16)

            with tc.tile_pool(name="pro", bufs=3) as pro, \
                 tc.tile_pool(name="psA", bufs=2, space="PSUM") as psA:
                # ---- phase B: u = x @ e_x  ->  u_pad DRAM ---------------
                ev = 0
                for b8 in range(BC):
                    urow = pro.tile([1, UPADW], BF16, tag="urow")
                    nc.vector.tensor_copy(urow[:, 0:512], zrow)
                    for th in range((T + 511) // 512):
                        w = min(512, T - th * 512)
                        ps = psA.tile([128, 512], F32, tag="ps")
                        nc.tensor.matmul(ps[0:1, :w], lhsT=ex_sb,
                                         rhs=xT_sb[:, b8 * T + th * 512:
                                                   b8 * T + th * 512 + w],
                                         start=True, stop=True)
                        dst = urow[:, 512 + th * 512:512 + th * 512 + w]
                        if ev % 2 == 0:
                            nc.scalar.copy(dst, ps[0:1, :w])
                        else:
                            nc.vector.tensor_copy(dst, ps[0:1, :w])
                        ev += 1
                    nc.sync.dma_start(out=upad_d[b8:b8 + 1, :], in_=urow)

                # ---- phase C: reversed shift matrix ---------------------
                for b8 in range(BC):
                    nc.sync.dma_start(
                        out=ushr[:, b8, :],
                        in_=_dap(upad_d, b8 * UPADW + 1, [[1, 128], [1, USHW]]))

                # ---- inline conv tile (0,32) ----------------------------
                tau0, tw = INLINE_TILE
                for b8 in range(BC):
                    for jt in range(4):
                        ps = psA.tile([128, 512], F32, tag="cps")
                        qi0 = tau0 + 384
                        nc.tensor.matmul(
                            ps[:, 0:tw], lhsT=g_sb[:, 0, jt * 128:(jt + 1) * 128],
                            rhs=ushr[:, b8, qi0:qi0 + tw],
                            start=True, stop=False)
                        nc.tensor.matmul(
                            ps[:, 0:tw], lhsT=wx_sb[:, jt * 128:(jt + 1) * 128],
                            rhs=xT_sb[:, b8 * T + tau0:b8 * T + tau0 + tw],
                            start=False, stop=True)
                        nc.vector.tensor_copy(c_sb[:, jt, tau0:tau0 + tw, b8],
                                              ps[:, 0:tw])

            # ---- static PSUM banks ---------------------------------------
            psum_stack = ExitStack()
            psH_pool = psum_stack.enter_context(
                tc.tile_pool(name="psH", bufs=1, space="PSUM"))
            psD2_pool = psum_stack.enter_context(
                tc.tile_pool(name="psD2", bufs=1, space="PSUM"))
            psH = [psH_pool.tile([128, 4, 128], F32, name=f"psh{i}")
                   for i in range(NB)]
            psD = [psD2_pool.tile([128, 512], F32, name=f"psd{i}")
                   for i in range(2)]

            # pre-create the Prelu bias const AP + act table outside the
            # critical section (const pool alloc inside crit is unsafe);
            # psH[0] holds garbage here, scratch is never read.
            nc.scalar.activation(scratch, psH[0][:, :, 0:BC],
                                 mybir.ActivationFunctionType.Prelu, alpha=0.2)

            # ---- semaphores ---------------------------------------------
            S_mm = nc.alloc_semaphore("s_mm")      # +1 per h step (last MM)
            S_act = nc.alloc_semaphore("s_act")    # +1 per ACT
            S_cv = nc.alloc_semaphore("s_cv")      # +1 per conv copy
            S_mmD = nc.alloc_semaphore("s_mmD")    # +1 per conv group final MM
            S_nop = nc.alloc_semaphore("s_nop")    # dummy update: walrus
            # requires any instruction with an attached wait to also carry
            # an update

            # ---- drip work bookkeeping ----------------------------------
            groups = []
            for (gt0, gtw) in DRIP_TILES:
                for b8 in range(BC):
                    for jt in range(4):
                        groups.append((gt0, gtw, b8, jt))
            tile_last_group = {}
            gidx = 0
            for ti in range(len(DRIP_TILES)):
                gidx += BC * 4
                tile_last_group[ti] = gidx - 1

            def cov_target(s):
                """copies needed before the c[..., s] identity matmul."""
                if s < DRIP_TILES[0][0]:
                    return 0
                for ti, (gt0, gtw) in enumerate(DRIP_TILES):
                    if gt0 <= s < gt0 + gtw:
                        return tile_last_group[ti] + 1
                raise AssertionError(s)

            def conv_mm_list(g):
                gt0, gtw, b8, jt = groups[g]
                k_hi = min(KCN, (gt0 + gtw - 1) // 128 + 1)
                return [("lag", kc) for kc in range(k_hi)] + [("wx", None)]

            mm_cost = {32: 70.0, 64: 95.0, 128: 150.0, 256: 260.0}

            def emit_conv_mm(g, kind, kc, is_first, is_last):
                gt0, gtw, b8, jt = groups[g]
                ps = psD[g % 2]
                if kind == "lag":
                    qi0 = gt0 + 384 - 128 * kc
                    ins = nc.tensor.matmul(
                        ps[:, 0:gtw],
                        lhsT=g_sb[:, kc, jt * 128:(jt + 1) * 128],
                        rhs=ushr[:, b8, qi0:qi0 + gtw],
                        start=is_first, stop=False, skip_group_check=True)
                else:
                    ins = nc.tensor.matmul(
                        ps[:, 0:gtw],
                        lhsT=wx_sb[:, jt * 128:(jt + 1) * 128],
                        rhs=xT_sb[:, b8 * T + gt0:b8 * T + gt0 + gtw],
                        start=False, stop=True, skip_group_check=True)
                if is_first and g >= 2:
                    # bank g%2 reused: wait for copy of group g-2
                    ins._wait_ge(S_cv, g - 1)
                    if not is_last:
                        ins.then_inc(S_nop)
                if is_last:
                    ins.then_inc(S_mmD)

            def emit_conv_copy(g):
                gt0, gtw, b8, jt = groups[g]
                ins = nc.vector.tensor_copy(c_sb[:, jt, gt0:gt0 + gtw, b8],
                                            psD[g % 2][:, 0:gtw])
                ins._wait_ge(S_mmD, g + 1)
                ins.then_inc(S_cv)

            # static drip schedule + feasibility check
            work = []
            for g in range(len(groups)):
                mms = conv_mm_list(g)
                for i, (kind, kc) in enumerate(mms):
                    work.append((g, kind, kc, i == 0, i == len(mms) - 1,
                                 mm_cost[groups[g][1]]))
            wpos = 0
            copy_sched = {}   # step -> [group,...]
            copy_step = {}
            for t in range(T):
                b = DRIP_BUDGET
                while wpos < len(work) and b > 0:
                    g, kind, kc, first, last, cost = work[wpos]
                    b -= cost
                    if last:
                        copy_sched.setdefault(t + 1, []).append(g)
                        copy_step[g] = t + 1
                    wpos += 1
            assert wpos == len(work), "drip did not finish within T steps"
            for ti, (gt0, gtw) in enumerate(DRIP_TILES):
                done = max(copy_step[g] for g in
                           range(tile_last_group[ti] - BC * 4 + 1,
                                 tile_last_group[ti] + 1))
                deadline = gt0 - NB
                assert done <= deadline, \
                    f"tile {ti} ({gt0},{gtw}) copies at step {done} > {deadline}"

            # ---- the critical h-loop ------------------------------------
            with tc.tile_critical(name="hloop"):
                wpos = 0
                for t in range(T):
                    ps = psH[t % NB]
                    # PE: 4 identity matmuls land c in PSUM (start=True).
                    # Gated only on conv coverage of c[..., t]; bank reuse
                    # is implied by step t-1's S_act>=t-1 wait + in-order
                    # Scalar queue (NB >= 2).
                    cv = cov_target(t)
                    for mc in range(4):
                        # start=True resets the WHOLE bank, so only the
                        # first identity matmul may carry it
                        ins = nc.tensor.matmul(
                            ps[:, mc, 0:BC],
                            lhsT=eye_sb,
                            rhs=c_sb[:, mc, t, :],
                            start=(mc == 0), stop=False,
                            skip_group_check=True)
                        if mc == 0 and cv > 0:
                            ins._wait_ge(S_cv, cv)
                            ins.then_inc(S_nop)
                    # PE: 16 W_h matmuls accumulate
                    first = True
                    for kc in range(4):
                        rhs = (h0[:, kc, :] if t == 0
                               else hb[:, t - 1, kc, 0:BC])
                        for mc in range(4):
                            ins = nc.tensor.matmul(
                                ps[:, mc, 0:BC],
                                lhsT=whT_sb[:, kc, mc * 128:(mc + 1) * 128],
                                rhs=rhs,
                                start=False, stop=(kc == 3),
                                skip_group_check=True)
                            if first:
                                if t > 0:
                                    ins._wait_ge(S_act, t)
                                    ins.then_inc(S_nop)
                                first = False
                            if kc == 3 and mc == 3:
                                ins.then_inc(S_mm)
                    # Scalar: activation
                    a = nc.scalar.activation(
                        hb[:, t], ps[:, :, 0:BC],
                        mybir.ActivationFunctionType.Prelu, alpha=0.2)
                    a._wait_ge(S_mm, t + 1)
                    a.then_inc(S_act)
                    # DVE: conv copies due this step
                    for g in copy_sched.get(t, []):
                        emit_conv_copy(g)
                    # PE: drip conv matmuls into the idle shadow
                    b = DRIP_BUDGET
                    while wpos < len(work) and b > 0:
                        g, kind, kc, first_mm, last_mm, cost = work[wpos]
                        emit_conv_mm(g, kind, kc, first_mm, last_mm)
                        b -= cost
                        wpos += 1
                    # SP: block output DMA
                    if (t + 1) % tblk == 0:
                        blk = (t + 1) // tblk - 1
                        nc.sync.wait_ge(S_act, (blk + 1) * tblk)
                        nc.sync.dma_start(
                            out=out_d[:, blk * tblk * 4 * BC:
                                      (blk + 1) * tblk * 4 * BC],
                            in_=hb[:, blk * tblk:(blk + 1) * tblk],
                        ).then_inc(S_nop, 16)

            psum_stack.close()
            nc.clear_and_free_semaphores([S_mm, S_act, S_cv, S_mmD, S_nop])
    nc.compile()
    return nc


_nc_cache = {}


def _get_nc(T, tblk):
    key = (T, tblk)
    if key not in _nc_cache:
        _nc_cache[key] = build_nc(T, tblk)
    return _nc_cache[key]


def host_prep(A, Bv, W_x, e_x, W_h, W_m, T):
    order = A.shape[0]
    A64 = A.astype(np.float64)
    b64 = Bv[:, 0].astype(np.float64)
    Hk = np.empty((T, order))
    v = b64.copy()
    for k in range(T):
        Hk[k] = v
        v = A64 @ v
    G = (Hk @ W_m.T.astype(np.float64)).astype(np.float32)      # (T, 512)
    Gr = G.reshape(T // 128, 128, -1)[:, ::-1, :].reshape(T, -1)
    return (Gr.astype(BF), np.ascontiguousarray(W_h.T).astype(BF),
            W_x.astype(BF), e_x.astype(BF))


def kernel(x, A, Bv, W_x, e_x, W_h, W_m, T=TFULL, tblk=TBLK):
    x = np.asarray(x, np.float32)
    Gr, whT, wx16, ex16 = host_prep(
        np.asarray(A, np.float32), np.asarray(Bv, np.float32),
        np.asarray(W_x, np.float32), np.asarray(e_x, np.float32),
        np.asarray(W_h, np.float32), np.asarray(W_m, np.float32), T)

    nc = _get_nc(T, tblk)
    B = x.shape[0]
    eye = np.eye(128, dtype=np.float32).astype(BF)
    in_maps = []
    for c in range(NCORES):
        xs = x[c * BC:(c + 1) * BC, 1:T + 1, :].reshape(BC * T, FEAT)
        xst = np.ascontiguousarray(xs.astype(BF).T)   # [feat, (b,tau)]
        in_maps.append({
            "xt": xst, "whT": whT, "g": Gr, "wx": wx16, "ex": ex16,
            "eye": eye,
        })
    trace = bool(int(os.environ.get("KERNEL_TRACE", "0")))
    res = run_bass_kernel_spmd(nc, in_maps, list(range(NCORES)), trace=trace)
    last_run_info.clear()
    last_run_info.update(
        exec_time_ns=res.exec_time_ns,
        mean_exec_time_ns=res.mean_exec_time_ns,
        profile_json=res.profile_json,
    )
    out = np.empty((B, T, HID), np.float32)
    for c in range(NCORES):
        raw = res.results[c]["out"].reshape(128, T, 4, BC)
        o = raw.transpose(3, 1, 2, 0).reshape(BC, T, HID)
        out[c * BC:(c + 1) * BC] = o.astype(np.float32)
    return out


if __name__ == "__main__":
    # build-only smoke test
    nc = build_nc()
    print("build ok")


# revision 3
# speedup vs baseline: 1.0032x; 1.0032x over previous
# Trainium2 Bass kernel for the KerasLMU problem — v10 (v5 + u/ushr DMAs on the gpsimd queue).
#
# Math: per time step t (T=1024),
#   u_t = x_t @ e_x                       (B,1)
#   m_t = m_{t-1} @ A.T + b_row * u_t     (B,256)   -- linear recurrence
#   h_t = lrelu(x_t @ W_x + h_{t-1} @ W_h.T + m_t @ W_m.T)
#
# m is folded into a causal convolution with precomputed G[k] = W_m @ (A^k b),
# so c_t := x_t @ W_x + m_t @ W_m.T is parallel work; only
# h_t = lrelu(c_t + h_{t-1} @ W_h.T) is sequential.
#
# v3 vs v2: the h-loop (and the dripped conv) runs inside a tile_critical
# section with hand-placed semaphores:
#   - only the last matmul of each step carries a sem update (v2 paid ~15ns
#     of PE sequencer time per matmul for the auto queue-counter updates);
#   - the c base lands in PSUM via 4 identity matmuls (start=True) at the
#     head of each step instead of a DVE cast, so the whole step is PE-only
#     and bank reuse is ordered for free (step t-1's wait on S_act>=t-1 plus
#     the in-order Scalar queue implies ACT(t-NB) completed);
#   - the full h output stays resident in SBUF (64KB/partition); 4 block
#     DMAs drain it asynchronously.
#
# Sharding: data-parallel over batch. 64 rows -> 8 cores x 8. No collectives.

import os
import sys
from contextlib import ExitStack

sys.path.insert(0, "/opt/trn_rl_repo")

import numpy as np
import ml_dtypes

import concourse.bass as bass
import concourse.tile as tile
from concourse import bacc, mybir
from concourse.bass_utils import run_bass_kernel_spmd

F32 = mybir.dt.float32
BF16 = mybir.dt.bfloat16
BF = ml_dtypes.bfloat16

NCORES = 8
BATCH = 64
BC = BATCH // NCORES          # batch rows per core = 8
FEAT = 128
HID = 512
ORDER = 256
TFULL = 1024
TBLK = 256                    # steps per output DMA block

NB = 4                        # h-loop PSUM banks (rotating)
DRIP_BUDGET = 330.0           # ns of conv matmul work dripped per h step

last_run_info = {}


def _dap(handle, offset, dims):
    base = handle[:]
    return bass.AP(tensor=base.tensor, offset=offset, ap=[list(d) for d in dims])


def build_nc(T=TFULL, tblk=TBLK):
    assert T % tblk == 0
    BT = BC * T
    KCN = T // 128

    nc = bacc.Bacc(None, target_bir_lowering=False)
    xt_d = nc.declare_dram_parameter("xt", [FEAT, BT], BF16, isOutput=False)
    whT_d = nc.declare_dram_parameter("whT", [HID, HID], BF16, isOutput=False)
    g_d = nc.declare_dram_parameter("g", [T, HID], BF16, isOutput=False)
    wx_d = nc.declare_dram_parameter("wx", [FEAT, HID], BF16, isOutput=False)
    ex_d = nc.declare_dram_parameter("ex", [FEAT, 1], BF16, isOutput=False)
    eye_d = nc.declare_dram_parameter("eye", [128, 128], BF16, isOutput=False)
    # raw h dump: [p, t, mc, b] — contiguous per partition
    out_d = nc.declare_dram_parameter("out", [128, T * 4 * BC], BF16,
                                      isOutput=True)

    UPADW = 512 + T
    upad_d = nc.dram_tensor("u_pad", [BC, UPADW], BF16)
    USHW = T + 384

    # conv tau tiles: (0,32) runs inline pre-loop; the rest drip into the
    # h-loop's PE idle window. All drip tiles <=256 cols so a single drip
    # matmul never exceeds the per-step shadow.
    INLINE_TILE = (0, 32)
    DRIP_TILES = [(32, 32), (64, 64), (128, 128), (256, 256),
                  (512, 256), (768, 256)]

    with tile.TileContext(nc) as tc:
        with tc.tile_pool(name="consts", bufs=1) as consts:
            # ---- resident constants (DMA order: ex/xT first — phase B
            # depends on them; g last, only needed by inline conv) --------
            ex_sb = consts.tile([128, 1], BF16)
            nc.sync.dma_start(out=ex_sb, in_=ex_d[:, :])
            xT_sb = consts.tile([128, BT], BF16)      # x.T : [feat, (b,tau)]
            for b8 in range(BC):
                nc.sync.dma_start(out=xT_sb[:, b8 * T:(b8 + 1) * T],
                                  in_=xt_d[:, b8 * T:(b8 + 1) * T])
            wx_sb = consts.tile([128, HID], BF16)
            nc.sync.dma_start(out=wx_sb, in_=wx_d[:, :])
            g_sb = consts.tile([128, KCN, HID], BF16)
            for kc in range(KCN):
                nc.sync.dma_start(out=g_sb[:, kc, :],
                                  in_=g_d[kc * 128:(kc + 1) * 128, :])
            eye_sb = consts.tile([128, 128], BF16)
            nc.sync.dma_start(out=eye_sb, in_=eye_d[:, :])
            whT_sb = consts.tile([128, 4, HID], BF16)
            for kc in range(4):
                nc.sync.dma_start(out=whT_sb[:, kc, :],
                                  in_=whT_d[kc * 128:(kc + 1) * 128, :])

            ushr = consts.tile([128, BC, USHW], BF16)  # reversed u shifts
            c_sb = consts.tile([128, 4, T, BC], BF16)  # c.T resident
            hb = consts.tile([128, T, 4, BC], BF16)    # full h output resident
            zrow = consts.tile([1, 512], BF16)
            nc.vector.memset(zrow, 0.0)
            h0 = consts.tile([128, 4, BC], BF16)
            nc.vector.memset(h0, 0.0)
            scratch = consts.tile([128, 4, BC], BF16)

            with tc.tile_pool(name="pro", bufs=3) as pro, \
                 tc.tile_pool(name="psA", bufs=2, space="PSUM") as psA:
                # ---- phase B: u = x @ e_x  ->  u_pad DRAM ---------------
                ev = 0
                for b8 in range(BC):
                    urow = pro.tile([1, UPADW], BF16, tag="urow")
                    nc.vector.tensor_copy(urow[:, 0:512], zrow)
                    for th in range((T + 511) // 512):
                        w = min(512, T - th * 512)
                        ps = psA.tile([128, 512], F32, tag="ps")
                        nc.tensor.matmul(ps[0:1, :w], lhsT=ex_sb,
                                         rhs=xT_sb[:, b8 * T + th * 512:
                                                   b8 * T + th * 512 + w],
                                         start=True, stop=True)
                        dst = urow[:, 512 + th * 512:512 + th * 512 + w]
                        if ev % 2 == 0:
                            nc.scalar.copy(dst, ps[0:1, :w])
                        else:
                            nc.vector.tensor_copy(dst, ps[0:1, :w])
                        ev += 1
                    # gpsimd DGE queue: bypasses the SP queue's ~5.5MB of
                    # const loads so ushr is ready before the drip needs it
                    nc.gpsimd.dma_start(out=upad_d[b8:b8 + 1, :], in_=urow)

                # ---- phase C: reversed shift matrix ---------------------
                for b8 in range(BC):
                    nc.gpsimd.dma_start(
                        out=ushr[:, b8, :],
                        in_=_dap(upad_d, b8 * UPADW + 1, [[1, 128], [1, USHW]]))

                # ---- inline conv tile (0,32) ----------------------------
                tau0, tw = INLINE_TILE
                for b8 in range(BC):
                    for jt in range(4):
                        ps = psA.tile([128, 512], F32, tag="cps")
                        qi0 = tau0 + 384
                        nc.tensor.matmul(
                            ps[:, 0:tw], lhsT=g_sb[:, 0, jt * 128:(jt + 1) * 128],
                            rhs=ushr[:, b8, qi0:qi0 + tw],
                            start=True, stop=False)
                        nc.tensor.matmul(
                            ps[:, 0:tw], lhsT=wx_sb[:, jt * 128:(jt + 1) * 128],
                            rhs=xT_sb[:, b8 * T + tau0:b8 * T + tau0 + tw],
                            start=False, stop=True)
                        nc.vector.tensor_copy(c_sb[:, jt, tau0:tau0 + tw, b8],
                                              ps[:, 0:tw])

            # ---- static PSUM banks ---------------------------------------
            psum_stack = ExitStack()
            psH_pool = psum_stack.enter_context(
                tc.tile_pool(name="psH", bufs=1, space="PSUM"))
            psD2_pool = psum_stack.enter_context(
                tc.tile_pool(name="psD2", bufs=1, space="PSUM"))
            psH = [psH_pool.tile([128, 4, 128], F32, name=f"psh{i}")
                   for i in range(NB)]
            psD = [psD2_pool.tile([128, 512], F32, name=f"psd{i}")
                   for i in range(2)]

            # pre-create the Prelu bias const AP + act table outside the
            # critical section (const pool alloc inside crit is unsafe);
            # psH[0] holds garbage here, scratch is never read.
            nc.scalar.activation(scratch, psH[0][:, :, 0:BC],
                                 mybir.ActivationFunctionType.Prelu, alpha=0.2)

            # ---- semaphores ---------------------------------------------
            S_mm = nc.alloc_semaphore("s_mm")      # +1 per h step (last MM)
            S_act = nc.alloc_semaphore("s_act")    # +1 per ACT
            S_cv = nc.alloc_semaphore("s_cv")      # +1 per conv copy
            S_mmD = nc.alloc_semaphore("s_mmD")    # +1 per conv group final MM
            S_nop = nc.alloc_semaphore("s_nop")    # dummy update: walrus
            # requires any instruction with an attached wait to also carry
            # an update

            # ---- drip work bookkeeping ----------------------------------
            groups = []
            for (gt0, gtw) in DRIP_TILES:
                for b8 in range(BC):
                    for jt in range(4):
                        groups.append((gt0, gtw, b8, jt))
            tile_last_group = {}
            gidx = 0
            for ti in range(len(DRIP_TILES)):
                gidx += BC * 4
                tile_last_group[ti] = gidx - 1

            def cov_target(s):
                """copies needed before the c[..., s] identity matmul."""
                if s < DRIP_TILES[0][0]:
                    return 0
                for ti, (gt0, gtw) in enumerate(DRIP_TILES):
                    if gt0 <= s < gt0 + gtw:
                        return tile_last_group[ti] + 1
                raise AssertionError(s)

            def conv_mm_list(g):
                gt0, gtw, b8, jt = groups[g]
                k_hi = min(KCN, (gt0 + gtw - 1) // 128 + 1)
                return [("lag", kc) for kc in range(k_hi)] + [("wx", None)]

            mm_cost = {32: 70.0, 64: 95.0, 128: 150.0, 256: 260.0}

            def emit_conv_mm(g, kind, kc, is_first, is_last):
                gt0, gtw, b8, jt = groups[g]
                ps = psD[g % 2]
                if kind == "lag":
                    qi0 = gt0 + 384 - 128 * kc
                    ins = nc.tensor.matmul(
                        ps[:, 0:gtw],
                        lhsT=g_sb[:, kc, jt * 128:(jt + 1) * 128],
                        rhs=ushr[:, b8, qi0:qi0 + gtw],
                        start=is_first, stop=False, skip_group_check=True)
                else:
                    ins = nc.tensor.matmul(
                        ps[:, 0:gtw],
                        lhsT=wx_sb[:, jt * 128:(jt + 1) * 128],
                        rhs=xT_sb[:, b8 * T + gt0:b8 * T + gt0 + gtw],
                        start=False, stop=True, skip_group_check=True)
                if is_first and g >= 2:
                    # bank g%2 reused: wait for copy of group g-2
                    ins._wait_ge(S_cv, g - 1)
                    if not is_last:
                        ins.then_inc(S_nop)
                if is_last:
                    ins.then_inc(S_mmD)

            def emit_conv_copy(g):
                gt0, gtw, b8, jt = groups[g]
                ins = nc.vector.tensor_copy(c_sb[:, jt, gt0:gt0 + gtw, b8],
                                            psD[g % 2][:, 0:gtw])
                ins._wait_ge(S_mmD, g + 1)
                ins.then_inc(S_cv)

            # static drip schedule + feasibility check
            work = []
            for g in range(len(groups)):
                mms = conv_mm_list(g)
                for i, (kind, kc) in enumerate(mms):
                    work.append((g, kind, kc, i == 0, i == len(mms) - 1,
                                 mm_cost[groups[g][1]]))
            wpos = 0
            copy_sched = {}   # step -> [group,...]
            copy_step = {}
            for t in range(T):
                b = DRIP_BUDGET
                while wpos < len(work) and b > 0:
                    g, kind, kc, first, last, cost = work[wpos]
                    b -= cost
                    if last:
                        copy_sched.setdefault(t + 1, []).append(g)
                        copy_step[g] = t + 1
                    wpos += 1
            assert wpos == len(work), "drip did not finish within T steps"
            for ti, (gt0, gtw) in enumerate(DRIP_TILES):
                done = max(copy_step[g] for g in
                           range(tile_last_group[ti] - BC * 4 + 1,
                                 tile_last_group[ti] + 1))
                deadline = gt0 - NB
                assert done <= deadline, \
                    f"tile {ti} ({gt0},{gtw}) copies at step {done} > {deadline}"

            # ---- the critical h-loop ------------------------------------
            with tc.tile_critical(name="hloop"):
                wpos = 0
                for t in range(T):
                    ps = psH[t % NB]
                    # PE: 4 identity matmuls land c in PSUM (start=True).
                    # Gated only on conv coverage of c[..., t]; bank reuse
                    # is implied by step t-1's S_act>=t-1 wait + in-order
                    # Scalar queue (NB >= 2).
                    cv = cov_target(t)
                    for mc in range(4):
                        # start=True resets the WHOLE bank, so only the
                        # first identity matmul may carry it
                        ins = nc.tensor.matmul(
                            ps[:, mc, 0:BC],
                            lhsT=eye_sb,
                            rhs=c_sb[:, mc, t, :],
                            start=(mc == 0), stop=False,
                            skip_group_check=True)
                        if mc == 0 and cv > 0:
                            ins._wait_ge(S_cv, cv)
                            ins.then_inc(S_nop)
                    # PE: 16 W_h matmuls accumulate
                    first = True
                    for kc in range(4):
                        rhs = (h0[:, kc, :] if t == 0
                               else hb[:, t - 1, kc, 0:BC])
                        for mc in range(4):
                            ins = nc.tensor.matmul(
                                ps[:, mc, 0:BC],
                                lhsT=whT_sb[:, kc, mc * 128:(mc + 1) * 128],
                                rhs=rhs,
                                start=False, stop=(kc == 3),
                                skip_group_check=True)
                            if first:
                                if t > 0:
                                    ins._wait_ge(S_act, t)
                                    ins.then_inc(S_nop)
                                first = False
                            if kc == 3 and mc == 3:
                                ins.then_inc(S_mm)
                    # Scalar: activation
                    a = nc.scalar.activation(
                        hb[:, t], ps[:, :, 0:BC],
                        mybir.ActivationFunctionType.Prelu, alpha=0.2)
                    a._wait_ge(S_mm, t + 1)
                    a.then_inc(S_act)
                    # DVE: conv copies due this step
                    for g in copy_sched.get(t, []):
                        emit_conv_copy(g)
                    # PE: drip conv matmuls into the idle shadow
                    b = DRIP_BUDGET
                    while wpos < len(work) and b > 0:
                        g, kind, kc, first_mm, last_mm, cost = work[wpos]
                        emit_conv_mm(g, kind, kc, first_mm, last_mm)
                        b -= cost
                        wpos += 1
                    # SP: block output DMA
                    if (t + 1) % tblk == 0:
                        blk = (t + 1) // tblk - 1
                        nc.sync.wait_ge(S_act, (blk + 1) * tblk)
                        nc.sync.dma_start(
                            out=out_d[:, blk * tblk * 4 * BC:
                                      (blk + 1) * tblk * 4 * BC],
                            in_=hb[:, blk * tblk:(blk + 1) * tblk],
                        ).then_inc(S_nop, 16)

            psum_stack.close()
            nc.clear_and_free_semaphores([S_mm, S_act, S_cv, S_mmD, S_nop])
    nc.compile()
    return nc


_nc_cache = {}


def _get_nc(T, tblk):
    key = (T, tblk)
    if key not in _nc_cache:
        _nc_cache[key] = build_nc(T, tblk)
    return _nc_cache[key]


def host_prep(A, Bv, W_x, e_x, W_h, W_m, T):
    order = A.shape[0]
    A64 = A.astype(np.float64)
    b64 = Bv[:, 0].astype(np.float64)
    Hk = np.empty((T, order))
    v = b64.copy()
    for k in range(T):
        Hk[k] = v
        v = A64 @ v
    G = (Hk @ W_m.T.astype(np.float64)).astype(np.float32)      # (T, 512)
    Gr = G.reshape(T // 128, 128, -1)[:, ::-1, :].reshape(T, -1)
    return (Gr.astype(BF), np.ascontiguousarray(W_h.T).astype(BF),
            W_x.astype(BF), e_x.astype(BF))


def kernel(x, A, Bv, W_x, e_x, W_h, W_m, T=TFULL, tblk=TBLK):
    x = np.asarray(x, np.float32)
    Gr, whT, wx16, ex16 = host_prep(
        np.asarray(A, np.float32), np.asarray(Bv, np.float32),
        np.asarray(W_x, np.float32), np.asarray(e_x, np.float32),
        np.asarray(W_h, np.float32), np.asarray(W_m, np.float32), T)

    nc = _get_nc(T, tblk)
    B = x.shape[0]
    eye = np.eye(128, dtype=np.float32).astype(BF)
    in_maps = []
    for c in range(NCORES):
        xs = x[c * BC:(c + 1) * BC, 1:T + 1, :].reshape(BC * T, FEAT)
        xst = np.ascontiguousarray(xs.astype(BF).T)   # [feat, (b,tau)]
        in_maps.append({
            "xt": xst, "whT": whT, "g": Gr, "wx": wx16, "ex": ex16,
            "eye": eye,
        })
    trace = bool(int(os.environ.get("KERNEL_TRACE", "0")))
    res = run_bass_kernel_spmd(nc, in_maps, list(range(NCORES)), trace=trace)
    last_run_info.clear()
    last_run_info.update(
        exec_time_ns=res.exec_time_ns,
        mean_exec_time_ns=res.mean_exec_time_ns,
        profile_json=res.profile_json,
    )
    out = np.empty((B, T, HID), np.float32)
    for c in range(NCORES):
        raw = res.results[c]["out"].reshape(128, T, 4, BC)
        o = raw.transpose(3, 1, 2, 0).reshape(BC, T, HID)
        out[c * BC:(c + 1) * BC] = o.astype(np.float32)
    return out


if __name__ == "__main__":
    # build-only smoke test
    nc = build_nc()
    print("build ok")


# revision 4
# speedup vs baseline: 1.0082x; 1.0050x over previous
# Trainium2 Bass kernel for the KerasLMU problem — v12 (v10 + 4-deep phase-B pipeline).
#
# Math: per time step t (T=1024),
#   u_t = x_t @ e_x                       (B,1)
#   m_t = m_{t-1} @ A.T + b_row * u_t     (B,256)   -- linear recurrence
#   h_t = lrelu(x_t @ W_x + h_{t-1} @ W_h.T + m_t @ W_m.T)
#
# m is folded into a causal convolution with precomputed G[k] = W_m @ (A^k b),
# so c_t := x_t @ W_x + m_t @ W_m.T is parallel work; only
# h_t = lrelu(c_t + h_{t-1} @ W_h.T) is sequential.
#
# v3 vs v2: the h-loop (and the dripped conv) runs inside a tile_critical
# section with hand-placed semaphores:
#   - only the last matmul of each step carries a sem update (v2 paid ~15ns
#     of PE sequencer time per matmul for the auto queue-counter updates);
#   - the c base lands in PSUM via 4 identity matmuls (start=True) at the
#     head of each step instead of a DVE cast, so the whole step is PE-only
#     and bank reuse is ordered for free (step t-1's wait on S_act>=t-1 plus
#     the in-order Scalar queue implies ACT(t-NB) completed);
#   - the full h output stays resident in SBUF (64KB/partition); 4 block
#     DMAs drain it asynchronously.
#
# Sharding: data-parallel over batch. 64 rows -> 8 cores x 8. No collectives.

import os
import sys
from contextlib import ExitStack

sys.path.insert(0, "/opt/trn_rl_repo")

import numpy as np
import ml_dtypes

import concourse.bass as bass
import concourse.tile as tile
from concourse import bacc, mybir
from concourse.bass_utils import run_bass_kernel_spmd

F32 = mybir.dt.float32
BF16 = mybir.dt.bfloat16
BF = ml_dtypes.bfloat16

NCORES = 8
BATCH = 64
BC = BATCH // NCORES          # batch rows per core = 8
FEAT = 128
HID = 512
ORDER = 256
TFULL = 1024
TBLK = 256                    # steps per output DMA block

NB = 4                        # h-loop PSUM banks (rotating)
DRIP_BUDGET = 330.0           # ns of conv matmul work dripped per h step

last_run_info = {}


def _dap(handle, offset, dims):
    base = handle[:]
    return bass.AP(tensor=base.tensor, offset=offset, ap=[list(d) for d in dims])


def build_nc(T=TFULL, tblk=TBLK):
    assert T % tblk == 0
    BT = BC * T
    KCN = T // 128

    nc = bacc.Bacc(None, target_bir_lowering=False)
    xt_d = nc.declare_dram_parameter("xt", [FEAT, BT], BF16, isOutput=False)
    whT_d = nc.declare_dram_parameter("whT", [HID, HID], BF16, isOutput=False)
    g_d = nc.declare_dram_parameter("g", [T, HID], BF16, isOutput=False)
    wx_d = nc.declare_dram_parameter("wx", [FEAT, HID], BF16, isOutput=False)
    ex_d = nc.declare_dram_parameter("ex", [FEAT, 1], BF16, isOutput=False)
    eye_d = nc.declare_dram_parameter("eye", [128, 128], BF16, isOutput=False)
    # raw h dump: [p, t, mc, b] — contiguous per partition
    out_d = nc.declare_dram_parameter("out", [128, T * 4 * BC], BF16,
                                      isOutput=True)

    UPADW = 512 + T
    upad_d = nc.dram_tensor("u_pad", [BC, UPADW], BF16)
    USHW = T + 384

    # conv tau tiles: (0,32) runs inline pre-loop; the rest drip into the
    # h-loop's PE idle window. All drip tiles <=256 cols so a single drip
    # matmul never exceeds the per-step shadow.
    INLINE_TILE = (0, 32)
    DRIP_TILES = [(32, 32), (64, 64), (128, 128), (256, 256),
                  (512, 256), (768, 256)]

    with tile.TileContext(nc) as tc:
        with tc.tile_pool(name="consts", bufs=1) as consts:
            # ---- resident constants (DMA order: ex/xT first — phase B
            # depends on them; g last, only needed by inline conv) --------
            ex_sb = consts.tile([128, 1], BF16)
            nc.sync.dma_start(out=ex_sb, in_=ex_d[:, :])
            xT_sb = consts.tile([128, BT], BF16)      # x.T : [feat, (b,tau)]
            for b8 in range(BC):
                nc.sync.dma_start(out=xT_sb[:, b8 * T:(b8 + 1) * T],
                                  in_=xt_d[:, b8 * T:(b8 + 1) * T])
            wx_sb = consts.tile([128, HID], BF16)
            nc.sync.dma_start(out=wx_sb, in_=wx_d[:, :])
            g_sb = consts.tile([128, KCN, HID], BF16)
            for kc in range(KCN):
                nc.sync.dma_start(out=g_sb[:, kc, :],
                                  in_=g_d[kc * 128:(kc + 1) * 128, :])
            eye_sb = consts.tile([128, 128], BF16)
            nc.sync.dma_start(out=eye_sb, in_=eye_d[:, :])
            whT_sb = consts.tile([128, 4, HID], BF16)
            for kc in range(4):
                nc.sync.dma_start(out=whT_sb[:, kc, :],
                                  in_=whT_d[kc * 128:(kc + 1) * 128, :])

            ushr = consts.tile([128, BC, USHW], BF16)  # reversed u shifts
            c_sb = consts.tile([128, 4, T, BC], BF16)  # c.T resident
            hb = consts.tile([128, T, 4, BC], BF16)    # full h output resident
            zrow = consts.tile([1, 512], BF16)
            nc.vector.memset(zrow, 0.0)
            h0 = consts.tile([128, 4, BC], BF16)
            nc.vector.memset(h0, 0.0)
            scratch = consts.tile([128, 4, BC], BF16)

            with tc.tile_pool(name="pro", bufs=4) as pro, \
                 tc.tile_pool(name="psA", bufs=4, space="PSUM") as psA:
                # ---- phase B: u = x @ e_x  ->  u_pad DRAM ---------------
                ev = 0
                for b8 in range(BC):
                    urow = pro.tile([1, UPADW], BF16, tag="urow")
                    nc.vector.tensor_copy(urow[:, 0:512], zrow)
                    for th in range((T + 511) // 512):
                        w = min(512, T - th * 512)
                        ps = psA.tile([128, 512], F32, tag="ps")
                        nc.tensor.matmul(ps[0:1, :w], lhsT=ex_sb,
                                         rhs=xT_sb[:, b8 * T + th * 512:
                                                   b8 * T + th * 512 + w],
                                         start=True, stop=True)
                        dst = urow[:, 512 + th * 512:512 + th * 512 + w]
                        if ev % 2 == 0:
                            nc.scalar.copy(dst, ps[0:1, :w])
                        else:
                            nc.vector.tensor_copy(dst, ps[0:1, :w])
                        ev += 1
                    # gpsimd DGE queue: bypasses the SP queue's ~5.5MB of
                    # const loads so ushr is ready before the drip needs it
                    nc.gpsimd.dma_start(out=upad_d[b8:b8 + 1, :], in_=urow)

                # ---- phase C: reversed shift matrix ---------------------
                for b8 in range(BC):
                    nc.gpsimd.dma_start(
                        out=ushr[:, b8, :],
                        in_=_dap(upad_d, b8 * UPADW + 1, [[1, 128], [1, USHW]]))

                # ---- inline conv tile (0,32) ----------------------------
                tau0, tw = INLINE_TILE
                for b8 in range(BC):
                    for jt in range(4):
                        ps = psA.tile([128, 512], F32, tag="cps")
                        qi0 = tau0 + 384
                        nc.tensor.matmul(
                            ps[:, 0:tw], lhsT=g_sb[:, 0, jt * 128:(jt + 1) * 128],
                            rhs=ushr[:, b8, qi0:qi0 + tw],
                            start=True, stop=False)
                        nc.tensor.matmul(
                            ps[:, 0:tw], lhsT=wx_sb[:, jt * 128:(jt + 1) * 128],
                            rhs=xT_sb[:, b8 * T + tau0:b8 * T + tau0 + tw],
                            start=False, stop=True)
                        nc.vector.tensor_copy(c_sb[:, jt, tau0:tau0 + tw, b8],
                                              ps[:, 0:tw])

            # ---- static PSUM banks ---------------------------------------
            psum_stack = ExitStack()
            psH_pool = psum_stack.enter_context(
                tc.tile_pool(name="psH", bufs=1, space="PSUM"))
            psD2_pool = psum_stack.enter_context(
                tc.tile_pool(name="psD2", bufs=1, space="PSUM"))
            psH = [psH_pool.tile([128, 4, 128], F32, name=f"psh{i}")
                   for i in range(NB)]
            psD = [psD2_pool.tile([128, 512], F32, name=f"psd{i}")
                   for i in range(2)]

            # pre-create the Prelu bias const AP + act table outside the
            # critical section (const pool alloc inside crit is unsafe);
            # psH[0] holds garbage here, scratch is never read.
            nc.scalar.activation(scratch, psH[0][:, :, 0:BC],
                                 mybir.ActivationFunctionType.Prelu, alpha=0.2)

            # ---- semaphores ---------------------------------------------
            S_mm = nc.alloc_semaphore("s_mm")      # +1 per h step (last MM)
            S_act = nc.alloc_semaphore("s_act")    # +1 per ACT
            S_cv = nc.alloc_semaphore("s_cv")      # +1 per conv copy
            S_mmD = nc.alloc_semaphore("s_mmD")    # +1 per conv group final MM
            S_nop = nc.alloc_semaphore("s_nop")    # dummy update: walrus
            # requires any instruction with an attached wait to also carry
            # an update

            # ---- drip work bookkeeping ----------------------------------
            groups = []
            for (gt0, gtw) in DRIP_TILES:
                for b8 in range(BC):
                    for jt in range(4):
                        groups.append((gt0, gtw, b8, jt))
            tile_last_group = {}
            gidx = 0
            for ti in range(len(DRIP_TILES)):
                gidx += BC * 4
                tile_last_group[ti] = gidx - 1

            def cov_target(s):
                """copies needed before the c[..., s] identity matmul."""
                if s < DRIP_TILES[0][0]:
                    return 0
                for ti, (gt0, gtw) in enumerate(DRIP_TILES):
                    if gt0 <= s < gt0 + gtw:
                        return tile_last_group[ti] + 1
                raise AssertionError(s)

            def conv_mm_list(g):
                gt0, gtw, b8, jt = groups[g]
                k_hi = min(KCN, (gt0 + gtw - 1) // 128 + 1)
                return [("lag", kc) for kc in range(k_hi)] + [("wx", None)]

            mm_cost = {32: 70.0, 64: 95.0, 128: 150.0, 256: 260.0}

            def emit_conv_mm(g, kind, kc, is_first, is_last):
                gt0, gtw, b8, jt = groups[g]
                ps = psD[g % 2]
                if kind == "lag":
                    qi0 = gt0 + 384 - 128 * kc
                    ins = nc.tensor.matmul(
                        ps[:, 0:gtw],
                        lhsT=g_sb[:, kc, jt * 128:(jt + 1) * 128],
                        rhs=ushr[:, b8, qi0:qi0 + gtw],
                        start=is_first, stop=False, skip_group_check=True)
                else:
                    ins = nc.tensor.matmul(
                        ps[:, 0:gtw],
                        lhsT=wx_sb[:, jt * 128:(jt + 1) * 128],
                        rhs=xT_sb[:, b8 * T + gt0:b8 * T + gt0 + gtw],
                        start=False, stop=True, skip_group_check=True)
                if is_first and g >= 2:
                    # bank g%2 reused: wait for copy of group g-2
                    ins._wait_ge(S_cv, g - 1)
                    if not is_last:
                        ins.then_inc(S_nop)
                if is_last:
                    ins.then_inc(S_mmD)

            def emit_conv_copy(g):
                gt0, gtw, b8, jt = groups[g]
                ins = nc.vector.tensor_copy(c_sb[:, jt, gt0:gt0 + gtw, b8],
                                            psD[g % 2][:, 0:gtw])
                ins._wait_ge(S_mmD, g + 1)
                ins.then_inc(S_cv)

            # static drip schedule + feasibility check
            work = []
            for g in range(len(groups)):
                mms = conv_mm_list(g)
                for i, (kind, kc) in enumerate(mms):
                    work.append((g, kind, kc, i == 0, i == len(mms) - 1,
                                 mm_cost[groups[g][1]]))
            wpos = 0
            copy_sched = {}   # step -> [group,...]
            copy_step = {}
            for t in range(T):
                b = DRIP_BUDGET
                while wpos < len(work) and b > 0:
                    g, kind, kc, first, last, cost = work[wpos]
                    b -= cost
                    if last:
                        copy_sched.setdefault(t + 1, []).append(g)
                        copy_step[g] = t + 1
                    wpos += 1
            assert wpos == len(work), "drip did not finish within T steps"
            for ti, (gt0, gtw) in enumerate(DRIP_TILES):
                done = max(copy_step[g] for g in
                           range(tile_last_group[ti] - BC * 4 + 1,
                                 tile_last_group[ti] + 1))
                deadline = gt0 - NB
                assert done <= deadline, \
                    f"tile {ti} ({gt0},{gtw}) copies at step {done} > {deadline}"

            # ---- the critical h-loop ------------------------------------
            with tc.tile_critical(name="hloop"):
                wpos = 0
                for t in range(T):
                    ps = psH[t % NB]
                    # PE: 4 identity matmuls land c in PSUM (start=True).
                    # Gated only on conv coverage of c[..., t]; bank reuse
                    # is implied by step t-1's S_act>=t-1 wait + in-order
                    # Scalar queue (NB >= 2).
                    cv = cov_target(t)
                    for mc in range(4):
                        # start=True resets the WHOLE bank, so only the
                        # first identity matmul may carry it
                        ins = nc.tensor.matmul(
                            ps[:, mc, 0:BC],
                            lhsT=eye_sb,
                            rhs=c_sb[:, mc, t, :],
                            start=(mc == 0), stop=False,
                            skip_group_check=True)
                        if mc == 0 and cv > 0:
                            ins._wait_ge(S_cv, cv)
                            ins.then_inc(S_nop)
                    # PE: 16 W_h matmuls accumulate
                    first = True
                    for kc in range(4):
                        rhs = (h0[:, kc, :] if t == 0
                               else hb[:, t - 1, kc, 0:BC])
                        for mc in range(4):
                            ins = nc.tensor.matmul(
                                ps[:, mc, 0:BC],
                                lhsT=whT_sb[:, kc, mc * 128:(mc + 1) * 128],
                                rhs=rhs,
                                start=False, stop=(kc == 3),
                                skip_group_check=True)
                            if first:
                                if t > 0:
                                    ins._wait_ge(S_act, t)
                                    ins.then_inc(S_nop)
                                first = False
                            if kc == 3 and mc == 3:
                                ins.then_inc(S_mm)
                    # Scalar: activation
                    a = nc.scalar.activation(
                        hb[:, t], ps[:, :, 0:BC],
                        mybir.ActivationFunctionType.Prelu, alpha=0.2)
                    a._wait_ge(S_mm, t + 1)
                    a.then_inc(S_act)
                    # DVE: conv copies due this step
                    for g in copy_sched.get(t, []):
                        emit_conv_copy(g)
                    # PE: drip conv matmuls into the idle shadow
                    b = DRIP_BUDGET
                    while wpos < len(work) and b > 0:
                        g, kind, kc, first_mm, last_mm, cost = work[wpos]
                        emit_conv_mm(g, kind, kc, first_mm, last_mm)
                        b -= cost
                        wpos += 1
                    # SP: block output DMA
                    if (t + 1) % tblk == 0:
                        blk = (t + 1) // tblk - 1
                        nc.sync.wait_ge(S_act, (blk + 1) * tblk)
                        nc.sync.dma_start(
                            out=out_d[:, blk * tblk * 4 * BC:
                                      (blk + 1) * tblk * 4 * BC],
                            in_=hb[:, blk * tblk:(blk + 1) * tblk],
                        ).then_inc(S_nop, 16)

            psum_stack.close()
            nc.clear_and_free_semaphores([S_mm, S_act, S_cv, S_mmD, S_nop])
    nc.compile()
    return nc


_nc_cache = {}


def _get_nc(T, tblk):
    key = (T, tblk)
    if key not in _nc_cache:
        _nc_cache[key] = build_nc(T, tblk)
    return _nc_cache[key]


def host_prep(A, Bv, W_x, e_x, W_h, W_m, T):
    order = A.shape[0]
    A64 = A.astype(np.float64)
    b64 = Bv[:, 0].astype(np.float64)
    Hk = np.empty((T, order))
    v = b64.copy()
    for k in range(T):
        Hk[k] = v
        v = A64 @ v
    G = (Hk @ W_m.T.astype(np.float64)).astype(np.float32)      # (T, 512)
    Gr = G.reshape(T // 128, 128, -1)[:, ::-1, :].reshape(T, -1)
    return (Gr.astype(BF), np.ascontiguousarray(W_h.T).astype(BF),
            W_x.astype(BF), e_x.astype(BF))


def kernel(x, A, Bv, W_x, e_x, W_h, W_m, T=TFULL, tblk=TBLK):
    x = np.asarray(x, np.float32)
    Gr, whT, wx16, ex16 = host_prep(
        np.asarray(A, np.float32), np.asarray(Bv, np.float32),
        np.asarray(W_x, np.float32), np.asarray(e_x, np.float32),
        np.asarray(W_h, np.float32), np.asarray(W_m, np.float32), T)

    nc = _get_nc(T, tblk)
    B = x.shape[0]
    eye = np.eye(128, dtype=np.float32).astype(BF)
    in_maps = []
    for c in range(NCORES):
        xs = x[c * BC:(c + 1) * BC, 1:T + 1, :].reshape(BC * T, FEAT)
        xst = np.ascontiguousarray(xs.astype(BF).T)   # [feat, (b,tau)]
        in_maps.append({
            "xt": xst, "whT": whT, "g": Gr, "wx": wx16, "ex": ex16,
            "eye": eye,
        })
    trace = bool(int(os.environ.get("KERNEL_TRACE", "0")))
    res = run_bass_kernel_spmd(nc, in_maps, list(range(NCORES)), trace=trace)
    last_run_info.clear()
    last_run_info.update(
        exec_time_ns=res.exec_time_ns,
        mean_exec_time_ns=res.mean_exec_time_ns,
        profile_json=res.profile_json,
    )
    out = np.empty((B, T, HID), np.float32)
    for c in range(NCORES):
        raw = res.results[c]["out"].reshape(128, T, 4, BC)
        o = raw.transpose(3, 1, 2, 0).reshape(BC, T, HID)
        out[c * BC:(c + 1) * BC] = o.astype(np.float32)
    return out


if __name__ == "__main__":
    # build-only smoke test
    nc = build_nc()
    print("build ok")
